# revision 1
# baseline (speedup 1.0000x reference)
"""Trainium2 Bass kernel for nn_AbstractAttention (dense transformer attention
with RoPE, B=2 S=2048 D=4096 H=32), tensor-parallel over heads on 8 cores.

Self-contained: builds the Bass program, shards inputs on host, runs via
run_bass_kernel_spmd, and reduces the partial outputs on host.

Numerics: all big matmuls use a 3-term tf32 (float32r) split
x@w ~= xh@wh + xh@wl + xl@wh (hi/lo split on host or on-device via DVE
rounding), which matches fp32 accuracy at ~3/4 the PE cost. Attention
probs@V stays native fp32. Softmax skips the max-subtraction (scores are
O(1) by construction), so exp/sum run in a single pass; the softmax
denominator rides in an extra all-ones column of V.
"""

import math

import numpy as np

import concourse.bacc as bacc
import concourse.mybir as mybir
from concourse.bass_utils import run_bass_kernel_spmd
from concourse.masks import make_identity
from concourse.tile import TileContext

P = 128  # partitions / head_dim
F32 = mybir.dt.float32
BF16 = mybir.dt.bfloat16
F32R = mybir.dt.float32r
ALU = mybir.AluOpType
ACTF = mybir.ActivationFunctionType

# set by test.py for profiling; grading path leaves these alone
TRACE = False
TRACE_DIR = None
LAST_RESULT = [None]

ZERO, MIXED, SKIP = 0, 1, 2


def _tf32(a):
    u = np.ascontiguousarray(a, dtype=np.float32).view(np.uint32)
    r = ((u >> 13).astype(np.uint64) + ((u >> 12) & 1)) << 13
    return (r & 0xFFFFFFFF).astype(np.uint32).view(np.float32)


def _classify_blocks(maskT, S, QT):
    """maskT: [S, S] (k, q). Returns (kinds[nqt][nkc], mixed_tiles, mixed_index)."""
    nqt, nkc = S // QT, S // P
    kinds = [[ZERO] * nkc for _ in range(nqt)]
    tiles = []
    index = {}
    for qt in range(nqt):
        for kc in range(nkc):
            sub = maskT[kc * P : (kc + 1) * P, qt * QT : (qt + 1) * QT]
            if np.all(sub == 0.0):
                kinds[qt][kc] = ZERO
            elif np.all(np.isneginf(sub) | (sub < -1e30)):
                kinds[qt][kc] = SKIP
            else:
                kinds[qt][kc] = MIXED
                index[(qt, kc)] = len(tiles)
                # pre-scale by sqrt(HD): kernel computes exp((S + m)/sqrt(HD))
                m = np.maximum(sub.astype(np.float64) * math.sqrt(P), -1e30)
                tiles.append(m.astype(np.float32))
    if tiles:
        mixed = np.ascontiguousarray(np.stack(tiles)).astype(np.float32)
    else:
        mixed = np.zeros((1, P, QT), dtype=np.float32)
    return kinds, mixed, index


def _build(B, S, D, HL, kinds, mixed_index, n_mixed):
    """Build the per-core Bass program (HL local heads, DL=HL*128 local dims)."""
    DL = HL * P
    T = B * S
    NCH = T // P
    CHB = S // P
    QT = 512
    NQT = S // QT
    NKC = S // P
    KD = D // P
    inv_sqrt_d = 1.0 / math.sqrt(P)

    nc = bacc.Bacc(None, target_bir_lowering=False)

    xTh = nc.declare_dram_parameter("xTh", [D, T], F32R, isOutput=False)
    xThb = nc.declare_dram_parameter("xThb", [D, T], BF16, isOutput=False)
    xTlb = nc.declare_dram_parameter("xTlb", [D, T], BF16, isOutput=False)
    wqTh = nc.declare_dram_parameter("wqTh", [D, DL], F32R, isOutput=False)
    wqTlb = nc.declare_dram_parameter("wqTlb", [D, DL], BF16, isOutput=False)
    wqThb = nc.declare_dram_parameter("wqThb", [D, DL], BF16, isOutput=False)
    wkTh = nc.declare_dram_parameter("wkTh", [D, DL], F32R, isOutput=False)
    wkTlb = nc.declare_dram_parameter("wkTlb", [D, DL], BF16, isOutput=False)
    wkThb = nc.declare_dram_parameter("wkThb", [D, DL], BF16, isOutput=False)
    wvTh = nc.declare_dram_parameter("wvTh", [D, DL], F32R, isOutput=False)
    wvTlb = nc.declare_dram_parameter("wvTlb", [D, DL], BF16, isOutput=False)
    wvThb = nc.declare_dram_parameter("wvThb", [D, DL], BF16, isOutput=False)
    woTh = nc.declare_dram_parameter("woTh", [DL, D], F32R, isOutput=False)
    woTl = nc.declare_dram_parameter("woTl", [DL, D], F32R, isOutput=False)
    cos_e = nc.declare_dram_parameter("cos_e", [S, P], F32, isOutput=False)
    nsin_e = nc.declare_dram_parameter("nsin_e", [S, P], F32, isOutput=False)
    mtiles = nc.declare_dram_parameter(
        "mask_tiles", [max(n_mixed, 1), P, QT], F32, isOutput=False
    )
    out = nc.declare_dram_parameter("out", [T, D], F32, isOutput=True)

    ts = lambda i, s: slice(i * s, (i + 1) * s)

    with TileContext(nc) as tc:
        with (
            tc.tile_pool(name="consts", bufs=1) as consts,
            tc.tile_pool(name="stage", bufs=2) as stage,
            tc.tile_pool(name="small", bufs=4) as small,
            tc.tile_pool(name="dram", bufs=1, space="DRAM") as dram,
        ):
            ident = consts.tile([P, P], F32)
            make_identity(nc, ident)
            ones_h = consts.tile([P, HL], F32)
            nc.vector.memset(ones_h, 1.0)

            # DRAM scratch
            qT_scr = [
                [
                    dram.tile([P, HL, S], F32R, tag=f"qT{p}{b}", name=f"qT{p}{b}")
                    for p in range(2)
                ]
                for b in range(B)
            ]
            kT_scr = [
                [
                    dram.tile([P, HL, S], F32R, tag=f"kT{p}{b}", name=f"kT{p}{b}")
                    for p in range(2)
                ]
                for b in range(B)
            ]
            v_scr = [
                dram.tile([S, HL, P + 1], F32, tag=f"v{b}", name=f"v{b}")
                for b in range(B)
            ]
            cTh_scr = [
                dram.tile([DL, S], F32R, tag=f"cTh{b}", name=f"cTh{b}")
                for b in range(B)
            ]
            cTl_scr = [
                dram.tile([DL, S], F32R, tag=f"cTl{b}", name=f"cTl{b}")
                for b in range(B)
            ]

            xTh_r = xTh.ap().rearrange("(o p) t -> p o t", p=P)
            xThb_r = xThb.ap().rearrange("(o p) t -> p o t", p=P)
            xTlb_r = xTlb.ap().rearrange("(o p) t -> p o t", p=P)

            # ================= Phase 1: projections (+rope for q/k) ==========
            # K streams in quarters through double-buffered weight tiles while
            # 8 token-chunks accumulate in the 8 PSUM banks.
            with (
                tc.tile_pool(name="p1c", bufs=1) as p1c,
                tc.tile_pool(name="wpool", bufs=2) as wpool,
                tc.tile_pool(name="xpool", bufs=4) as xpool,
                tc.tile_pool(name="work", bufs=4) as work,
                tc.tile_pool(name="qsbp", bufs=4) as qsbp,
                tc.tile_pool(name="psA1", bufs=6, space="PSUM") as psA1,
                tc.tile_pool(name="psT1", bufs=2, space="PSUM") as psT1,
            ):
                cos_sb = p1c.tile([P, CHB, P], F32)
                nsin_sb = p1c.tile([P, CHB, P], F32)
                nc.sync.dma_start(
                    cos_sb, cos_e.ap().rearrange("(o p) d -> p o d", p=P)
                )
                nc.sync.dma_start(
                    nsin_sb, nsin_e.ap().rearrange("(o p) d -> p o d", p=P)
                )

                NQTR = 8
                KQ = KD // NQTR

                def proj_pass(wTh, wTlb, wThb, rope, name):
                    CG = 6
                    wh_r = wTh.ap().rearrange("(o p) n -> p o n", p=P)
                    wlb_r = wTlb.ap().rearrange("(o p) n -> p o n", p=P)
                    whb_r = wThb.ap().rearrange("(o p) n -> p o n", p=P)
                    groups = [
                        list(range(s, min(s + CG, NCH)))
                        for s in range(0, NCH, CG)
                    ]

                    def emit_mms(grp):
                        banks = {}
                        for qtr in range(NQTR):
                            w_h = wpool.tile([P, KQ, DL], F32R, tag="wh")
                            nc.scalar.dma_start(w_h, wh_r[:, ts(qtr, KQ)])
                            w_lb = wpool.tile([P, KQ, DL], BF16, tag="wlb")
                            nc.scalar.dma_start(w_lb, wlb_r[:, ts(qtr, KQ)])
                            w_hb = wpool.tile([P, KQ, DL], BF16, tag="whb")
                            nc.scalar.dma_start(w_hb, whb_r[:, ts(qtr, KQ)])
                            for cp in range((len(grp) + 1) // 2):
                                ch0 = grp[0] + cp * 2
                                x_h = xpool.tile([P, KQ, 2 * P], F32R, tag="xh")
                                nc.sync.dma_start(
                                    x_h,
                                    xTh_r[:, ts(qtr, KQ), ts(ch0 // 2, 2 * P)],
                                )
                                x_hb = xpool.tile([P, KQ, 2 * P], BF16, tag="xhb")
                                nc.sync.dma_start(
                                    x_hb,
                                    xThb_r[:, ts(qtr, KQ), ts(ch0 // 2, 2 * P)],
                                )
                                x_lb = xpool.tile([P, KQ, 2 * P], BF16, tag="xlb")
                                nc.sync.dma_start(
                                    x_lb,
                                    xTlb_r[:, ts(qtr, KQ), ts(ch0 // 2, 2 * P)],
                                )
                                for sub in range(min(2, len(grp) - cp * 2)):
                                    ci = cp * 2 + sub
                                    if qtr == 0:
                                        banks[ci] = psA1.tile(
                                            [P, DL], F32, tag="ps_a",
                                            name=f"psb{ci}",
                                        )
                                    ps = banks[ci]
                                    i = 0
                                    for a, w in [
                                        (x_h, w_h), (x_hb, w_lb), (x_lb, w_hb)
                                    ]:
                                        for j in range(KQ):
                                            nc.tensor.matmul(
                                                ps,
                                                a[:, j, ts(sub, P)],
                                                w[:, j, :],
                                                start=(qtr == 0 and i == 0),
                                                stop=(
                                                    qtr == NQTR - 1
                                                    and i == 3 * KQ - 1
                                                ),
                                            )
                                            i += 1
                        # release PSUM banks promptly via ACT copies
                        held = []
                        for ci in range(len(grp)):
                            ch = grp[ci]
                            b, so = ch // CHB, ch % CHB
                            if not rope:
                                vst = stage.tile([P, HL, P + 1], F32, tag="vst")
                                nc.vector.memset(vst[:, :, P : P + 1], 1.0)
                                nc.scalar.copy(
                                    vst[:, :, 0:P],
                                    banks[ci].rearrange("p (h d) -> p h d", h=HL),
                                )
                                nc.sync.dma_start(v_scr[b][ts(so, P), :, :], vst)
                            else:
                                qsb = qsbp.tile([P, DL], F32, tag="qsb")
                                nc.scalar.copy(qsb, banks[ci])
                                held.append((ch, qsb))
                        return held

                    def emit_tail(held):
                        scr = qT_scr if name == "q" else kT_scr
                        for ch, qsb in held:
                            b, so = ch // CHB, ch % CHB
                            cosv = cos_sb[:, so, None, :].to_broadcast((P, HL, P))
                            t1 = work.tile([P, DL], F32, tag="t1")
                            t1v = t1.rearrange("p (h d) -> p h d", h=HL)
                            qv = qsb.rearrange("p (h d) -> p h d", h=HL)
                            nc.vector.tensor_tensor(t1v, qv, cosv, ALU.mult)
                            t2 = work.tile([P, DL], F32, tag="t2")
                            qs4 = qsb.rearrange(
                                "p (h n two) -> p h n two", h=HL, two=2
                            )
                            t24 = t2.rearrange(
                                "p (h n two) -> p h n two", h=HL, two=2
                            )
                            nsv = nsin_sb[:, so, :].rearrange(
                                "p (n two) -> p n two", two=2
                            )
                            nc.vector.tensor_tensor(
                                t24[:, :, :, 0:1],
                                qs4[:, :, :, 1:2],
                                nsv[:, None, :, 0:1].to_broadcast(
                                    (P, HL, P // 2, 1)
                                ),
                                ALU.mult,
                            )
                            nc.vector.tensor_tensor(
                                t24[:, :, :, 1:2],
                                qs4[:, :, :, 0:1],
                                nsv[:, None, :, 1:2].to_broadcast(
                                    (P, HL, P // 2, 1)
                                ),
                                ALU.mult,
                            )
                            nc.vector.tensor_tensor(t1, t1, t2, ALU.add)
                            t1v = t1.rearrange("p (h d) -> p h d", h=HL)
                            pt = psT1.tile([P, DL], F32, tag="ptb")
                            for h in range(HL):
                                nc.tensor.matmul(
                                    pt[:, ts(h, P)],
                                    t1v[:, h, :],
                                    ident,
                                    is_transpose=True,
                                    start=(h == 0),
                                    stop=(h == HL - 1),
                                )
                            sth = stage.tile([P, DL], F32R, tag="trh")
                            nc.vector.tensor_copy(sth, pt)
                            stl = stage.tile([P, DL], F32R, tag="trl")
                            nc.vector.tensor_tensor(stl, pt, sth, ALU.subtract)
                            sthv = sth.rearrange("p (h t) -> p h t", h=HL)
                            stlv = stl.rearrange("p (h t) -> p h t", h=HL)
                            nc.sync.dma_start(scr[b][0][:, :, ts(so, P)], sthv)
                            nc.sync.dma_start(scr[b][1][:, :, ts(so, P)], stlv)

                    pending = None
                    for grp in groups:
                        held = emit_mms(grp)
                        if pending:
                            emit_tail(pending)
                        pending = held
                    if pending:
                        emit_tail(pending)

                proj_pass(wqTh, wqTlb, wqThb, True, "q")
                proj_pass(wkTh, wkTlb, wkThb, True, "k")
                proj_pass(wvTh, wvTlb, wvThb, False, "v")

            # ================= Phase 2: attention ============================
            active = [
                [kc for kc in range(NKC) if kinds[qt][kc] != SKIP]
                for qt in range(NQT)
            ]
            with (
                tc.tile_pool(name="wop", bufs=2, side="right") as wop,
                tc.tile_pool(name="mpool", bufs=1) as mpool,
                tc.tile_pool(name="a2", bufs=1) as a2,
                tc.tile_pool(name="qtp", bufs=2) as qtp,
                tc.tile_pool(name="work2", bufs=3) as work2,
                tc.tile_pool(name="cpool", bufs=2) as cpool,
                tc.tile_pool(name="psA2", bufs=3, space="PSUM") as psA2,
                tc.tile_pool(name="psCtx", bufs=1, space="PSUM") as psCtx,
                tc.tile_pool(name="psT2", bufs=1, space="PSUM") as psT2,
            ):
                # wo weight halves prefetch during attention (right-side pool)
                OH = D // 4
                wo_tiles = []
                woh_r = woTh.ap().rearrange("(o p) n -> p o n", p=P)
                wol_r = woTl.ap().rearrange("(o p) n -> p o n", p=P)

                def load_wo_quarter(ohalf):
                    wo_h = wop.tile([P, HL, OH], F32R, tag="woh",
                                    name=f"woh{ohalf}")
                    nc.gpsimd.dma_start(wo_h, woh_r[:, :, ts(ohalf, OH)])
                    wo_l = wop.tile([P, HL, OH], F32R, tag="wol",
                                    name=f"wol{ohalf}")
                    nc.gpsimd.dma_start(wo_l, wol_r[:, :, ts(ohalf, OH)])
                    wo_tiles.append((wo_h, wo_l))

                mcache = {}
                if n_mixed <= 20:
                    for (qt, kc), idx in mixed_index.items():
                        mt = mpool.tile(
                            [P, QT], F32, tag=f"m{qt}_{kc}", name=f"m{qt}_{kc}"
                        )
                        nc.sync.dma_start(mt, mtiles.ap()[idx])
                        mcache[(qt, kc)] = mt

                nbh = B * HL
                wo_load_at = [max(1, ((i + 1) * nbh) // 5) for i in range(4)]
                for b in range(B):
                    for h in range(HL):
                        bhi = b * HL + h
                        while len(wo_tiles) < 4 and wo_load_at[len(wo_tiles)] <= bhi:
                            load_wo_quarter(len(wo_tiles))
                        kth = a2.tile([P, S], F32R, tag="kth")
                        nc.scalar.dma_start(kth, kT_scr[b][0][:, h, :])
                        ktl = a2.tile([P, S], F32R, tag="ktl")
                        nc.scalar.dma_start(ktl, kT_scr[b][1][:, h, :])
                        v_sb = a2.tile([P, NKC, P + 1], F32, tag="v_sb")
                        nc.scalar.dma_start(
                            v_sb,
                            v_scr[b][:, h, :].rearrange("(o p) c -> p o c", p=P),
                        )
                        for qt in range(NQT):
                            acts = active[qt]
                            if not acts:
                                continue
                            qth = qtp.tile([P, QT], F32R, tag="qth")
                            nc.scalar.dma_start(
                                qth, qT_scr[b][0][:, h, ts(qt, QT)]
                            )
                            qtl = qtp.tile([P, QT], F32R, tag="qtl")
                            nc.scalar.dma_start(
                                qtl, qT_scr[b][1][:, h, ts(qt, QT)]
                            )
                            ctx_ps = [
                                psCtx.tile(
                                    [P, P + 1], F32, tag=f"ctx{i}", name=f"ctx{i}"
                                )
                                for i in range(QT // P)
                            ]
                            for ki, kc in enumerate(acts):
                                sps = psA2.tile([P, QT], F32, tag="ps_a2")
                                for ti, (lh, rh) in enumerate(
                                    [(kth, qth), (kth, qtl), (ktl, qth)]
                                ):
                                    nc.tensor.matmul(
                                        sps,
                                        lh[:, ts(kc, P)],
                                        rh,
                                        start=(ti == 0),
                                        stop=(ti == 2),
                                    )
                                pT = work2.tile([P, QT], F32, tag="pT")
                                if kinds[qt][kc] == MIXED:
                                    if (qt, kc) in mcache:
                                        mt = mcache[(qt, kc)]
                                    else:
                                        mt = work2.tile([P, QT], F32, tag="mload")
                                        nc.sync.dma_start(
                                            mt, mtiles.ap()[mixed_index[(qt, kc)]]
                                        )
                                    nc.vector.tensor_tensor(sps, sps, mt, ALU.add)
                                nc.scalar.activation(
                                    pT, sps, ACTF.Exp, scale=inv_sqrt_d
                                )
                                for qsi in range(QT // P):
                                    nc.tensor.matmul(
                                        ctx_ps[qsi],
                                        pT[:, ts(qsi, P)],
                                        v_sb[:, kc, :],
                                        start=(ki == 0),
                                        stop=(ki == len(acts) - 1),
                                    )
                            for qsi in range(QT // P):
                                rec = small.tile([P, 1], F32, tag="rec")
                                nc.vector.reciprocal(rec, ctx_ps[qsi][:, P : P + 1])
                                cn = stage.tile([P, P], F32, tag="cn")
                                nc.vector.tensor_scalar_mul(
                                    cn, ctx_ps[qsi][:, 0:P], rec
                                )
                                pt = psT2.tile([P, P], F32, tag="ptr2")
                                nc.tensor.transpose(pt, cn, ident)
                                chi = stage.tile([P, P], F32R, tag="chi")
                                nc.scalar.copy(chi, pt)
                                clo = stage.tile([P, P], F32R, tag="clo")
                                nc.vector.tensor_tensor(clo, pt, chi, ALU.subtract)
                                qcol = ts(qt * (QT // P) + qsi, P)
                                nc.sync.dma_start(cTh_scr[b][ts(h, P), qcol], chi)
                                nc.sync.dma_start(cTl_scr[b][ts(h, P), qcol], clo)

                while len(wo_tiles) < 4:
                    load_wo_quarter(len(wo_tiles))

                # ============= Phase 3: output projection =====================
                for ohalf in range(4):
                    wo_h, wo_l = wo_tiles[ohalf]
                    for b in range(B):
                        ch_r = cTh_scr[b].rearrange("(o p) t -> p o t", p=P)
                        cl_r = cTl_scr[b].rearrange("(o p) t -> p o t", p=P)
                        for c in range(CHB):
                            cTh = cpool.tile([P, HL, P], F32R, tag="cth3")
                            nc.sync.dma_start(cTh, ch_r[:, :, ts(c, P)])
                            cTl = cpool.tile([P, HL, P], F32R, tag="ctl3")
                            nc.sync.dma_start(cTl, cl_r[:, :, ts(c, P)])
                            for oc in range(OH // 512):
                                ps = psA2.tile([P, 512], F32, tag="ps_a2")
                                i = 0
                                for lh, rh in [
                                    (cTh, wo_h), (cTh, wo_l), (cTl, wo_h)
                                ]:
                                    for j in range(HL):
                                        nc.tensor.matmul(
                                            ps,
                                            lh[:, j, :],
                                            rh[:, j, ts(oc, 512)],
                                            start=(i == 0),
                                            stop=(i == 3 * HL - 1),
                                        )
                                        i += 1
                                st = stage.tile([P, 512], F32, tag="ost")
                                nc.scalar.copy(st, ps)
                                nc.sync.dma_start(
                                    out.ap()[
                                        ts(b * CHB + c, P),
                                        ts(ohalf * (OH // 512) + oc, 512),
                                    ],
                                    st,
                                )

    nc.finalize()
    return nc


def kernel(x, wq, wk, wv, wo, cos, sin, mask):
    B, S, D = x.shape
    H = D // P
    NCORES = 8
    HL = H // NCORES
    DL = HL * P
    T = B * S

    import ml_dtypes

    x = np.asarray(x, dtype=np.float32)
    xT = np.ascontiguousarray(x.reshape(T, D).T)
    xTh = _tf32(xT)
    xThb = xTh.astype(ml_dtypes.bfloat16)
    xTlb = (xT - xTh).astype(ml_dtypes.bfloat16)
    cos = np.asarray(cos, dtype=np.float32)
    sin = np.asarray(sin, dtype=np.float32)
    cos_e = np.repeat(cos, 2, axis=1).astype(np.float32)  # [S, 128]
    nsin_e = np.empty((S, P), dtype=np.float32)
    nsin_e[:, 0::2] = -sin
    nsin_e[:, 1::2] = sin

    maskT = np.ascontiguousarray(np.asarray(mask, dtype=np.float32)[0, 0].T)
    kinds, mixed, mixed_index = _classify_blocks(maskT, S, 512)

    nc = _build(B, S, D, HL, kinds, mixed_index, len(mixed))

    wq = np.asarray(wq, dtype=np.float32)
    wk = np.asarray(wk, dtype=np.float32)
    wv = np.asarray(wv, dtype=np.float32)
    wo = np.asarray(wo, dtype=np.float32)

    in_maps = []
    for c in range(NCORES):
        sl = slice(c * DL, (c + 1) * DL)
        m = {
            "xTh": xTh,
            "xThb": xThb,
            "xTlb": xTlb,
            "cos_e": cos_e,
            "nsin_e": nsin_e,
            "mask_tiles": mixed,
        }
        for nm, w in [("wq", wq), ("wk", wk), ("wv", wv)]:
            wt = np.ascontiguousarray(w[sl, :].T)
            wh = _tf32(wt)
            m[nm + "Th"] = wh
            m[nm + "Tlb"] = (wt - wh).astype(ml_dtypes.bfloat16)
            m[nm + "Thb"] = wh.astype(ml_dtypes.bfloat16)
        wot = np.ascontiguousarray(wo[:, sl].T)
        m["woTh"] = _tf32(wot)
        m["woTl"] = _tf32(wot - m["woTh"])
        in_maps.append(m)

    kwargs = {}
    if TRACE:
        kwargs = {"trace": True}
        if TRACE_DIR:
            kwargs["tmpdir"] = TRACE_DIR
    res = run_bass_kernel_spmd(nc, in_maps, core_ids=list(range(NCORES)), **kwargs)
    LAST_RESULT[0] = res

    acc = res.results[0]["out"].astype(np.float64)
    for c in range(1, NCORES):
        acc += res.results[c]["out"]
    return acc.astype(np.float32).reshape(B, S, D)



# revision 11
# speedup vs baseline: 2.8331x; 2.8331x over previous
"""Trainium2 Bass kernel for nn_AbstractAttention (dense transformer attention
with RoPE, B=2 S=2048 D=4096 H=32), tensor-parallel over heads on 8 cores.

Single-pass bf16 design (rel tolerance is 2e-2; end-to-end bf16 gives ~5e-3):
every logical matmul is ONE bf16 PE pass (vs the 3-term fp32-split baseline),
and all matmuls are emitted in "flipped" orientations so no PE transposes are
needed anywhere:

  q^T/k^T [dl, t]  = (wT chunk)^T @ x^T      (weight stationary)
  v [t, e]         = (x^T chunk)^T @ wvT     (x stationary)
  s^T [k, q]       = (k^T chunk)^T @ q^T     (k stationary)
  ctx^T [hd, q]    = (v chunk)^T @ p^T       (v stationary)  <- accumulates
  out^T [e, t]     = (woT chunk)^T @ ctx^T   (wo stationary)

RoPE is applied on the [hd-partition, token-free] layout: the pair swap is a
constant 128x128 permutation matmul on the PE; cos/sin become full [128, S]
tables (host-precomputed, sign+mask folded in) so the mix is 3 DVE ops.

Softmax skips max-subtraction (scores are O(1)); the denominator is a
Pool-engine accumulation of exp tiles + a ones-row matmul partition-reduce,
and normalization happens on the small context (not the probs) via a rank-1
broadcast matmul of the reciprocal.

Host: shards weights by head, preps bf16 transposed layouts, sums the 8
partial out^T tensors in fp32 and transposes back.
"""

import math

import numpy as np

import concourse.bacc as bacc
import concourse.mybir as mybir
from concourse.bass_utils import run_bass_kernel_spmd
from concourse.tile import TileContext

P = 128
F32 = mybir.dt.float32
BF16 = mybir.dt.bfloat16
ALU = mybir.AluOpType
ACTF = mybir.ActivationFunctionType

# set by test.py for profiling; grading path leaves these alone
TRACE = False
TRACE_DIR = None
LAST_RESULT = [None]

ZERO, MIXED, SKIP = 0, 1, 2


def _classify_blocks(maskT, S, QT):
    """maskT: [S, S] (k, q). Diagonal-straddling blocks get explicit tiles."""
    nqt, nkc = S // QT, S // P
    kinds = [[ZERO] * nkc for _ in range(nqt)]
    tiles = []
    index = {}
    for qt in range(nqt):
        for kc in range(nkc):
            sub = maskT[kc * P : (kc + 1) * P, qt * QT : (qt + 1) * QT]
            if np.all(sub == 0.0):
                kinds[qt][kc] = ZERO
            elif np.all(np.isneginf(sub) | (sub < -1e30)):
                kinds[qt][kc] = SKIP
            else:
                kinds[qt][kc] = MIXED
                index[(qt, kc)] = len(tiles)
                # pre-scale by sqrt(HD): kernel computes exp((S + m)/sqrt(HD))
                m = np.maximum(sub.astype(np.float64) * math.sqrt(P), -1e30)
                tiles.append(m.astype(np.float32))
    if tiles:
        mixed = np.ascontiguousarray(np.stack(tiles)).astype(np.float32)
    else:
        mixed = np.zeros((1, P, QT), dtype=np.float32)
    return kinds, mixed, index


def _p2_tail(nc, psden, apool, cpool, ones_mat, acc, ctx_ps, ctx_scr_r, h, tcol):
    """Deferred per-(b,h,qt) softmax-denominator + context normalization."""
    P = 128
    F32 = mybir.dt.float32
    BF16 = mybir.dt.bfloat16
    den = psden.tile([P, 512], F32, tag="den")
    nc.tensor.matmul(den, ones_mat, acc, start=True, stop=True)
    rec = apool.tile([P, 512], F32, tag="rec")
    scr = apool.tile([P, 512], F32, tag="rscr")
    nc.vector.reciprocal_approx_accurate(rec, den, scr)
    csb = cpool.tile([P, 512], BF16, tag="csb")
    nc.vector.tensor_tensor(csb, ctx_ps, rec, mybir.AluOpType.mult)
    nc.sync.dma_start(ctx_scr_r[:, h, tcol * 512 : (tcol + 1) * 512], csb)


def _build(B, S, D, HL, kinds, mixed_index, n_mixed):
    """Per-core Bass program. HL local heads, DL=HL*128 local dims."""
    DL = HL * P
    T = B * S
    KD = D // P        # 32 K-chunks of the model dim
    NTG = T // 512     # 8 token groups of 512
    NKC = S // P       # 16 key chunks per batch
    NQT = S // 512     # 4 query tiles per batch
    inv_sqrt_d = 1.0 / math.sqrt(P)

    nc = bacc.Bacc(None, target_bir_lowering=False)

    xT = nc.declare_dram_parameter("xT", [D, T], BF16, isOutput=False)
    wqT = nc.declare_dram_parameter("wqT", [D, DL], BF16, isOutput=False)
    wkT = nc.declare_dram_parameter("wkT", [D, DL], BF16, isOutput=False)
    wvT = nc.declare_dram_parameter("wvT", [D, DL], BF16, isOutput=False)
    woT = nc.declare_dram_parameter("woT", [DL, D], BF16, isOutput=False)
    cosE = nc.declare_dram_parameter("cosE", [P, S], BF16, isOutput=False)
    sinS = nc.declare_dram_parameter("sinS", [P, S], BF16, isOutput=False)
    pswap = nc.declare_dram_parameter("pswap", [P, P], BF16, isOutput=False)
    mtiles = nc.declare_dram_parameter(
        "mask_tiles", [max(n_mixed, 1), P, 512], F32, isOutput=False
    )
    outT = nc.declare_dram_parameter("outT", [D, T], BF16, isOutput=True)

    ts = lambda i, s: slice(i * s, (i + 1) * s)

    xT_r = xT.ap().rearrange("(o p) t -> p o t", p=P)
    woT_r = woT.ap().rearrange("(o p) e -> p o e", p=P)
    outT_r = outT.ap().rearrange("(o p) t -> p o t", p=P)

    with TileContext(nc) as tc:
        with (
            tc.tile_pool(name="res", bufs=1) as res,
            tc.tile_pool(name="consts", bufs=1) as consts,
            tc.tile_pool(name="dram", bufs=1, space="DRAM") as dram,
        ):
            # resident q^T/k^T (roped, bf16) and v
            qres = res.tile([P, HL, T], BF16)
            kres = res.tile([P, HL, T], BF16)
            vres = res.tile([P, T // P, DL], BF16)  # [k-in-chunk, t-chunk, (h,hd)]
            ctx_scr = dram.tile([DL, T], BF16, tag="ctxs", name="ctxs")
            ctx_scr_r = ctx_scr.rearrange("(o p) t -> p o t", p=P)

            ones_mat = consts.tile([P, P], F32)
            nc.vector.memset(ones_mat, 1.0)

            # ============ Phase 1: projections, order v -> q -> k ============
            # One rotating weights pool (quarter-tiles for fine-grained DMA
            # deps) so the next projection's weights prefetch during the
            # current one's matmuls.
            with (
                tc.tile_pool(name="p1c", bufs=1) as p1c,
                tc.tile_pool(name="wpool", bufs=2) as wpool,
                tc.tile_pool(name="xpool", bufs=3) as xpool,
                tc.tile_pool(name="rp", bufs=3) as rp,
                tc.tile_pool(name="psacc", bufs=6, space="PSUM") as psacc,
                tc.tile_pool(name="pssw", bufs=2, space="PSUM") as pssw,
            ):
                cos_sb = p1c.tile([P, S], BF16)
                nc.scalar.dma_start(cos_sb, cosE.ap())
                sin_sb = p1c.tile([P, S], BF16)
                nc.scalar.dma_start(sin_sb, sinS.ap())
                psw_sb = p1c.tile([P, P], BF16)
                nc.scalar.dma_start(psw_sb, pswap.ap())

                def load_w(src):
                    qtr = []
                    r = src.ap().rearrange("(o p) n -> p o n", p=P)
                    for kq in range(4):
                        t = wpool.tile([P, 8, DL], BF16, tag=f"w{kq}")
                        nc.sync.dma_start(t, r[:, ts(kq, 8), :])
                        qtr.append(t)
                    return qtr

                wv_sb = load_w(wvT)
                wq_sb = load_w(wqT)

                # --- v pass (x stationary) ---
                for tg in range(NTG):
                    vbanks = [
                        psacc.tile([P, DL], F32, tag="acc", name=f"vac{tg}{u}")
                        for u in range(4)
                    ]
                    for kq in range(4):
                        xt = xpool.tile([P, 8, 512], BF16, tag="xt")
                        nc.sync.dma_start(xt, xT_r[:, ts(kq, 8), ts(tg, 512)])
                        for u in range(4):
                            for kc in range(8):
                                nc.tensor.matmul(
                                    vbanks[u],
                                    xt[:, kc, ts(u, P)],
                                    wv_sb[kq][:, kc, :],
                                    start=(kq == 0 and kc == 0),
                                    stop=(kq == 3 and kc == 7),
                                )
                    for u in range(4):
                        nc.scalar.copy(vres[:, tg * 4 + u, :], vbanks[u])

                # --- q/k passes (weight stationary + rope) ---
                wk_sb = load_w(wkT)
                for proj in range(2):
                    w_sb = wq_sb if proj == 0 else wk_sb
                    dst = qres if proj == 0 else kres
                    for tg in range(NTG):
                        banks = [
                            psacc.tile([P, 512], F32, tag="acc", name=f"ac{proj}{tg}{d}")
                            for d in range(HL)
                        ]
                        for kq in range(4):
                            xt = xpool.tile([P, 8, 512], BF16, tag="xt")
                            nc.sync.dma_start(
                                xt, xT_r[:, ts(kq, 8), ts(tg, 512)]
                            )
                            for d in range(HL):
                                for kc in range(8):
                                    nc.tensor.matmul(
                                        banks[d],
                                        w_sb[kq][:, kc, ts(d, P)],
                                        xt[:, kc, :],
                                        start=(kq == 0 and kc == 0),
                                        stop=(kq == 3 and kc == 7),
                                    )
                        stg = tg % (S // 512)  # rope position repeats per batch
                        for d in range(HL):
                            # PSUM -> SBUF bf16 raw copy (frees the acc bank)
                            qraw = rp.tile([P, 512], BF16, tag="qraw")
                            nc.scalar.copy(qraw, banks[d])
                            # pair-swap via constant permutation matmul
                            qsw = pssw.tile([P, 512], F32, tag="sw")
                            nc.tensor.matmul(qsw, psw_sb, qraw, start=True, stop=True)
                            # rope mix: dst = raw*cos + swap*sin  (3 DVE ops)
                            dslc = dst[:, d, ts(tg, 512)]
                            nc.vector.tensor_tensor(
                                dslc, qraw, cos_sb[:, ts(stg, 512)], ALU.mult
                            )
                            tmp = rp.tile([P, 512], BF16, tag="rtmp")
                            nc.vector.tensor_tensor(
                                tmp, qsw, sin_sb[:, ts(stg, 512)], ALU.mult
                            )
                            nc.vector.tensor_tensor(dslc, dslc, tmp, ALU.add)

            # ============ Phase 2: attention ============
            active = [
                [kc for kc in range(NKC) if kinds[qt][kc] != SKIP]
                for qt in range(NQT)
            ]
            with tc.tile_pool(name="wop", bufs=2) as wop:
                # wo prefetch (first quarter) rides out phase 2
                wo_tiles = {}

                def load_wo(eq):
                    t = wop.tile([P, HL, 1024], BF16, tag="wo3")
                    nc.scalar.dma_start(t, woT_r[:, :, ts(eq, 1024)])
                    wo_tiles[eq] = t

                load_wo(0)

                with (
                    tc.tile_pool(name="mpool", bufs=1) as mpool,
                    tc.tile_pool(name="ppool", bufs=4) as ppool,
                    tc.tile_pool(name="apool", bufs=2) as apool,
                    tc.tile_pool(name="cpool", bufs=2) as cpool,
                    tc.tile_pool(name="pssc", bufs=3, space="PSUM") as pssc,
                    tc.tile_pool(name="psctx", bufs=2, space="PSUM") as psctx,
                    tc.tile_pool(name="psden", bufs=2, space="PSUM") as psden,
                ):
                    mcache = {}
                    for (qt, kc), idx in mixed_index.items():
                        mt = mpool.tile([P, 512], F32, tag=f"m{qt}_{kc}",
                                        name=f"m{qt}_{kc}")
                        nc.sync.dma_start(mt, mtiles.ap()[idx])
                        mcache[(qt, kc)] = mt

                    # body(i) emits scores/exp/PV; tail(i) (den/rec/norm/DMA)
                    # is emitted one iteration later so the den matmul never
                    # blocks the PE queue waiting on the Pool acc chain.
                    pending = None
                    for b in range(B):
                        for h in range(HL):
                            for qt in range(NQT):
                                acts = active[qt]
                                qslc = qres[:, h, ts(b * NQT + qt, 512)]
                                ctx_ps = psctx.tile([P, 512], F32, tag="ctx")
                                acc = apool.tile([P, 512], F32, tag="acc2")
                                for i, kc in enumerate(acts):
                                    sps = pssc.tile([P, 512], F32, tag="sc")
                                    nc.tensor.matmul(
                                        sps,
                                        kres[:, h, b * S + kc * P : b * S + (kc + 1) * P],
                                        qslc,
                                        start=True,
                                        stop=True,
                                    )
                                    if kinds[qt][kc] == MIXED:
                                        nc.vector.tensor_tensor(
                                            sps, sps, mcache[(qt, kc)], ALU.add
                                        )
                                    pT = ppool.tile([P, 512], BF16, tag="pT")
                                    nc.scalar.activation(
                                        pT, sps, ACTF.Exp, scale=inv_sqrt_d
                                    )
                                    nc.tensor.matmul(
                                        ctx_ps,
                                        vres[:, b * NKC + kc, ts(h, P)],
                                        pT,
                                        start=(i == 0),
                                        stop=(i == len(acts) - 1),
                                    )
                                    if i == 0:
                                        nc.gpsimd.tensor_copy(acc, pT)
                                    else:
                                        nc.gpsimd.tensor_tensor(
                                            acc, acc, pT, ALU.add
                                        )
                                if pending is not None:
                                    pending()
                                pending = (
                                    lambda b=b, h=h, qt=qt, acc=acc,
                                    ctx_ps=ctx_ps: _p2_tail(
                                        nc, psden, apool, cpool, ones_mat, acc,
                                        ctx_ps, ctx_scr_r, h, b * NQT + qt
                                    )
                                )
                    if pending is not None:
                        pending()

                # ===== Phase 3: output projection (wo stationary) =====
                with (
                    tc.tile_pool(name="cx3", bufs=3) as cx3,
                    tc.tile_pool(name="ost", bufs=3) as ost,
                    tc.tile_pool(name="pso", bufs=8, space="PSUM") as pso,
                ):
                    for eq in range(4):
                        if eq + 1 < 4:
                            load_wo(eq + 1)
                        wo_sb = wo_tiles.pop(eq)
                        for tg in range(NTG):
                            cxt = cx3.tile([P, HL, 512], BF16, tag="cx")
                            nc.sync.dma_start(
                                cxt, ctx_scr_r[:, :, ts(tg, 512)]
                            )
                            for es in range(8):
                                ps_o = pso.tile([P, 512], F32, tag="po")
                                for dl in range(HL):
                                    nc.tensor.matmul(
                                        ps_o,
                                        wo_sb[:, dl, ts(es, P)],
                                        cxt[:, dl, :],
                                        start=(dl == 0),
                                        stop=(dl == HL - 1),
                                    )
                                st = ost.tile([P, 512], BF16, tag="ost")
                                nc.scalar.copy(st, ps_o)
                                nc.sync.dma_start(
                                    outT_r[:, eq * 8 + es, ts(tg, 512)], st
                                )

    nc.finalize()
    return nc


def kernel(x, wq, wk, wv, wo, cos, sin, mask):
    B, S, D = x.shape
    H = D // P
    NCORES = 8
    HL = H // NCORES
    DL = HL * P
    T = B * S

    import ml_dtypes

    BF = ml_dtypes.bfloat16

    x = np.asarray(x, dtype=np.float32)
    xT = np.ascontiguousarray(x.reshape(T, D).T).astype(BF)
    cos = np.asarray(cos, dtype=np.float32)
    sin = np.asarray(sin, dtype=np.float32)

    # rope tables on [hd-partition, token-free] layout
    cosE = np.repeat(cos.T, 2, axis=0).astype(BF)          # [128, S]
    sinS = np.empty((P, S), dtype=np.float32)              # signed sin
    sinS[0::2] = -sin.T
    sinS[1::2] = sin.T
    sinS = sinS.astype(BF)
    pswap = np.zeros((P, P), dtype=np.float32)
    for r in range(P):
        pswap[r, r ^ 1] = 1.0
    pswap = pswap.astype(BF)

    maskT = np.ascontiguousarray(np.asarray(mask, dtype=np.float32)[0, 0].T)
    kinds, mixed, mixed_index = _classify_blocks(maskT, S, 512)

    nc = _build(B, S, D, HL, kinds, mixed_index, len(mixed))

    wq = np.asarray(wq, dtype=np.float32)
    wk = np.asarray(wk, dtype=np.float32)
    wv = np.asarray(wv, dtype=np.float32)
    wo = np.asarray(wo, dtype=np.float32)

    in_maps = []
    for c in range(NCORES):
        sl = slice(c * DL, (c + 1) * DL)
        m = {
            "xT": xT,
            "cosE": cosE,
            "sinS": sinS,
            "pswap": pswap,
            "mask_tiles": mixed,
            "wqT": np.ascontiguousarray(wq[sl, :].T).astype(BF),
            "wkT": np.ascontiguousarray(wk[sl, :].T).astype(BF),
            "wvT": np.ascontiguousarray(wv[sl, :].T).astype(BF),
            "woT": np.ascontiguousarray(wo[:, sl].T).astype(BF),
        }
        in_maps.append(m)

    kwargs = {}
    if TRACE:
        kwargs = {"trace": True}
        if TRACE_DIR:
            kwargs["tmpdir"] = TRACE_DIR
    res = run_bass_kernel_spmd(nc, in_maps, core_ids=list(range(NCORES)), **kwargs)
    LAST_RESULT[0] = res

    acc = res.results[0]["outT"].astype(np.float32)
    for c in range(1, NCORES):
        acc += res.results[c]["outT"].astype(np.float32)
    return np.ascontiguousarray(acc.T).reshape(B, S, D)


# revision 17
# speedup vs baseline: 3.4014x; 1.2006x over previous
"""Trainium2 Bass kernel for nn_AbstractAttention (dense transformer attention
with RoPE, B=2 S=2048 D=4096 H=32), tensor-parallel over heads on 8 cores.

Single-pass bf16 design (rel tolerance is 2e-2; end-to-end bf16 gives ~5e-3):
every logical matmul is ONE bf16 PE pass (vs the 3-term fp32-split baseline),
and all matmuls are emitted in "flipped" orientations so no PE transposes are
needed anywhere:

  q^T/k^T [dl, t]  = (wT chunk)^T @ x^T      (weight stationary)
  v [t, e]         = (x^T chunk)^T @ wvT     (x stationary)
  s^T [k, q]       = (k^T chunk)^T @ q^T     (k stationary)
  ctx^T [hd, q]    = (v chunk)^T @ p^T       (v stationary)  <- accumulates
  out^T [e, t]     = (woT chunk)^T @ ctx^T   (wo stationary)

RoPE is applied on the [hd-partition, token-free] layout: the pair swap is a
constant 128x128 permutation matmul on the PE; cos/sin become full [128, S]
tables (host-precomputed, sign+mask folded in) so the mix is 3 DVE ops.

Softmax skips max-subtraction (scores are O(1)); the denominator is a
Pool-engine accumulation of exp tiles + a ones-row matmul partition-reduce,
and normalization happens on the small context (not the probs) via a rank-1
broadcast matmul of the reciprocal.

Host: shards weights by head, preps bf16 transposed layouts, sums the 8
partial out^T tensors in fp32 and transposes back.
"""

import math

import numpy as np

import concourse.bacc as bacc
import concourse.mybir as mybir
from concourse.bass_utils import run_bass_kernel_spmd
from concourse.tile import TileContext

P = 128
F32 = mybir.dt.float32
BF16 = mybir.dt.bfloat16
ALU = mybir.AluOpType
ACTF = mybir.ActivationFunctionType

# set by test.py for profiling; grading path leaves these alone
TRACE = False
TRACE_DIR = None
LAST_RESULT = [None]

ZERO, MIXED, SKIP = 0, 1, 2


def _classify_blocks(maskT, S, QT):
    """maskT: [S, S] (k, q). Diagonal-straddling blocks get explicit tiles."""
    nqt, nkc = S // QT, S // P
    kinds = [[ZERO] * nkc for _ in range(nqt)]
    tiles = []
    index = {}
    for qt in range(nqt):
        for kc in range(nkc):
            sub = maskT[kc * P : (kc + 1) * P, qt * QT : (qt + 1) * QT]
            if np.all(sub == 0.0):
                kinds[qt][kc] = ZERO
            elif np.all(np.isneginf(sub) | (sub < -1e30)):
                kinds[qt][kc] = SKIP
            else:
                kinds[qt][kc] = MIXED
                index[(qt, kc)] = len(tiles)
                # pre-scale by sqrt(HD): kernel computes exp((S + m)/sqrt(HD))
                m = np.maximum(sub.astype(np.float64) * math.sqrt(P), -1e30)
                tiles.append(m.astype(np.float32))
    if tiles:
        mixed = np.ascontiguousarray(np.stack(tiles)).astype(np.float32)
    else:
        mixed = np.zeros((1, P, QT), dtype=np.float32)
    return kinds, mixed, index


def _p2_tail(nc, apool, cpool, den, ctx_ps, ctx_scr_r, h, tcol):
    """Deferred per-(b,h,qt) context normalization."""
    P = 128
    F32 = mybir.dt.float32
    BF16 = mybir.dt.bfloat16
    rec = apool.tile([P, 512], F32, tag="rec")
    scr = apool.tile([P, 512], F32, tag="rscr")
    nc.vector.reciprocal_approx_accurate(rec, den, scr)
    csb = cpool.tile([P, 512], BF16, tag="csb")
    nc.vector.tensor_tensor(csb, ctx_ps, rec, mybir.AluOpType.mult)
    nc.sync.dma_start(ctx_scr_r[:, h, tcol * 512 : (tcol + 1) * 512], csb)


def _build(B, S, D, HL, kinds, mixed_index, n_mixed):
    """Per-core Bass program. HL local heads, DL=HL*128 local dims."""
    DL = HL * P
    T = B * S
    KD = D // P        # 32 K-chunks of the model dim
    NTG = T // 512     # 8 token groups of 512
    NKC = S // P       # 16 key chunks per batch
    NQT = S // 512     # 4 query tiles per batch
    inv_sqrt_d = 1.0 / math.sqrt(P)

    nc = bacc.Bacc(None, target_bir_lowering=False)

    xT = nc.declare_dram_parameter("xT", [D, T], BF16, isOutput=False)
    wqT = nc.declare_dram_parameter("wqT", [D, DL], BF16, isOutput=False)
    wkT = nc.declare_dram_parameter("wkT", [D, DL], BF16, isOutput=False)
    wvT = nc.declare_dram_parameter("wvT", [D, DL], BF16, isOutput=False)
    woT = nc.declare_dram_parameter("woT", [DL, D], BF16, isOutput=False)
    cosE = nc.declare_dram_parameter("cosE", [P, S], BF16, isOutput=False)
    sinS = nc.declare_dram_parameter("sinS", [P, S], BF16, isOutput=False)
    pswap = nc.declare_dram_parameter("pswap", [P, P], BF16, isOutput=False)
    mtiles = nc.declare_dram_parameter(
        "mask_tiles", [max(n_mixed, 1), P, 512], F32, isOutput=False
    )
    outT = nc.declare_dram_parameter("outT", [D, T], BF16, isOutput=True)

    ts = lambda i, s: slice(i * s, (i + 1) * s)

    xT_r = xT.ap().rearrange("(o p) t -> p o t", p=P)
    woT_r = woT.ap().rearrange("(o p) e -> p o e", p=P)
    outT_r = outT.ap().rearrange("(o p) t -> p o t", p=P)

    with TileContext(nc) as tc:
        with (
            tc.tile_pool(name="res", bufs=1) as res,
            tc.tile_pool(name="consts", bufs=1) as consts,
            tc.tile_pool(name="dram", bufs=1, space="DRAM") as dram,
        ):
            # resident q^T/k^T (roped, bf16) and v
            qres = res.tile([P, HL, T], BF16)
            kres = res.tile([P, HL, T], BF16)
            vres = res.tile([P, T // P, DL], BF16)  # [k-in-chunk, t-chunk, (h,hd)]
            ctx_scr = dram.tile([DL, T], BF16, tag="ctxs", name="ctxs")
            ctx_scr_r = ctx_scr.rearrange("(o p) t -> p o t", p=P)

            ones_mat = consts.tile([P, P], BF16)
            nc.vector.memset(ones_mat, 1.0)

            # ============ Phase 1: projections, order v -> q -> k ============
            # One rotating weights pool (quarter-tiles for fine-grained DMA
            # deps) so the next projection's weights prefetch during the
            # current one's matmuls.
            with (
                tc.tile_pool(name="p1c", bufs=1) as p1c,
                tc.tile_pool(name="wpool", bufs=2) as wpool,
                tc.tile_pool(name="xpool", bufs=3) as xpool,
                tc.tile_pool(name="rp", bufs=3) as rp,
                tc.tile_pool(name="psacc", bufs=6, space="PSUM") as psacc,
                tc.tile_pool(name="pssw", bufs=2, space="PSUM") as pssw,
            ):
                cos_sb = p1c.tile([P, S], BF16)
                nc.scalar.dma_start(cos_sb, cosE.ap())
                sin_sb = p1c.tile([P, S], BF16)
                nc.scalar.dma_start(sin_sb, sinS.ap())
                psw_sb = p1c.tile([P, P], BF16)
                nc.scalar.dma_start(psw_sb, pswap.ap())

                def load_w(src):
                    # scalar-queue issue: keeps x tiles (sync queue) unblocked
                    qtr = []
                    r = src.ap().rearrange("(o p) n -> p o n", p=P)
                    for kq in range(4):
                        t = wpool.tile([P, 8, DL], BF16, tag=f"w{kq}")
                        nc.scalar.dma_start(t, r[:, ts(kq, 8), :])
                        qtr.append(t)
                    return qtr

                wv_sb = load_w(wvT)
                wq_sb = load_w(wqT)

                # --- v pass (x stationary) ---
                for tg in range(NTG):
                    vbanks = [
                        psacc.tile([P, DL], F32, tag="acc", name=f"vac{tg}{u}")
                        for u in range(4)
                    ]
                    for kq in range(4):
                        xt = xpool.tile([P, 8, 512], BF16, tag="xt")
                        nc.sync.dma_start(xt, xT_r[:, ts(kq, 8), ts(tg, 512)])
                        for u in range(4):
                            for kc in range(8):
                                nc.tensor.matmul(
                                    vbanks[u],
                                    xt[:, kc, ts(u, P)],
                                    wv_sb[kq][:, kc, :],
                                    start=(kq == 0 and kc == 0),
                                    stop=(kq == 3 and kc == 7),
                                )
                    for u in range(4):
                        nc.scalar.copy(vres[:, tg * 4 + u, :], vbanks[u])

                # --- q/k passes (weight stationary + rope) ---
                wk_sb = load_w(wkT)
                for proj in range(2):
                    w_sb = wq_sb if proj == 0 else wk_sb
                    dst = qres if proj == 0 else kres
                    for tg in range(NTG):
                        banks = [
                            psacc.tile([P, 512], F32, tag="acc", name=f"ac{proj}{tg}{d}")
                            for d in range(HL)
                        ]
                        for kq in range(4):
                            xt = xpool.tile([P, 8, 512], BF16, tag="xt")
                            nc.sync.dma_start(
                                xt, xT_r[:, ts(kq, 8), ts(tg, 512)]
                            )
                            for d in range(HL):
                                for kc in range(8):
                                    nc.tensor.matmul(
                                        banks[d],
                                        w_sb[kq][:, kc, ts(d, P)],
                                        xt[:, kc, :],
                                        start=(kq == 0 and kc == 0),
                                        stop=(kq == 3 and kc == 7),
                                    )
                        stg = tg % (S // 512)  # rope position repeats per batch
                        for d in range(HL):
                            # PSUM -> SBUF bf16 raw copy (frees the acc bank)
                            qraw = rp.tile([P, 512], BF16, tag="qraw")
                            nc.scalar.copy(qraw, banks[d])
                            # pair-swap via constant permutation matmul
                            qsw = pssw.tile([P, 512], F32, tag="sw")
                            nc.tensor.matmul(qsw, psw_sb, qraw, start=True, stop=True)
                            # rope mix: dst = raw*cos + swap*sin  (3 DVE ops)
                            dslc = dst[:, d, ts(tg, 512)]
                            nc.vector.tensor_tensor(
                                dslc, qraw, cos_sb[:, ts(stg, 512)], ALU.mult
                            )
                            tmp = rp.tile([P, 512], BF16, tag="rtmp")
                            nc.vector.tensor_tensor(
                                tmp, qsw, sin_sb[:, ts(stg, 512)], ALU.mult
                            )
                            nc.vector.tensor_tensor(dslc, dslc, tmp, ALU.add)

            # ============ Phase 2: attention ============
            active = [
                [kc for kc in range(NKC) if kinds[qt][kc] != SKIP]
                for qt in range(NQT)
            ]
            with tc.tile_pool(name="wop", bufs=1) as wop:
                # wo fully resident; DMA rides out during phase 2
                wo_sb = wop.tile([P, HL, D], BF16, tag="wo3")
                nc.scalar.dma_start(wo_sb, woT_r)

                with (
                    tc.tile_pool(name="mpool", bufs=1) as mpool,
                    tc.tile_pool(name="ppool", bufs=4) as ppool,
                    tc.tile_pool(name="apool", bufs=2) as apool,
                    tc.tile_pool(name="cpool", bufs=2) as cpool,
                    tc.tile_pool(name="pssc", bufs=3, space="PSUM") as pssc,
                    tc.tile_pool(name="psctx", bufs=2, space="PSUM") as psctx,
                    tc.tile_pool(name="psden", bufs=2, space="PSUM") as psden,
                ):
                    mcache = {}
                    for (qt, kc), idx in mixed_index.items():
                        mt = mpool.tile([P, 512], F32, tag=f"m{qt}_{kc}",
                                        name=f"m{qt}_{kc}")
                        nc.sync.dma_start(mt, mtiles.ap()[idx])
                        mcache[(qt, kc)] = mt

                    # body(i) emits scores/exp/PV; tail(i) (den/rec/norm/DMA)
                    # is emitted one iteration later so the den matmul never
                    # blocks the PE queue waiting on the Pool acc chain.
                    pending = None
                    for b in range(B):
                        for h in range(HL):
                            for qt in range(NQT):
                                acts = active[qt]
                                qslc = qres[:, h, ts(b * NQT + qt, 512)]
                                ctx_ps = psctx.tile([P, 512], F32, tag="ctx")
                                den = psden.tile([P, 512], F32, tag="den")
                                for i, kc in enumerate(acts):
                                    sps = pssc.tile([P, 512], F32, tag="sc")
                                    nc.tensor.matmul(
                                        sps,
                                        kres[:, h, b * S + kc * P : b * S + (kc + 1) * P],
                                        qslc,
                                        start=True,
                                        stop=True,
                                    )
                                    if kinds[qt][kc] == MIXED:
                                        nc.vector.tensor_tensor(
                                            sps, sps, mcache[(qt, kc)], ALU.add
                                        )
                                    pT = ppool.tile([P, 512], BF16, tag="pT")
                                    nc.scalar.activation(
                                        pT, sps, ACTF.Exp, scale=inv_sqrt_d
                                    )
                                    last = i == len(acts) - 1
                                    nc.tensor.matmul(
                                        ctx_ps,
                                        vres[:, b * NKC + kc, ts(h, P)],
                                        pT,
                                        start=(i == 0),
                                        stop=last,
                                    )
                                    # denominator rides the PE: every row of
                                    # den accumulates sum_k exp
                                    nc.tensor.matmul(
                                        den,
                                        ones_mat,
                                        pT,
                                        start=(i == 0),
                                        stop=last,
                                    )
                                if pending is not None:
                                    pending()
                                pending = (
                                    lambda b=b, h=h, qt=qt, den=den,
                                    ctx_ps=ctx_ps: _p2_tail(
                                        nc, apool, cpool, den,
                                        ctx_ps, ctx_scr_r, h, b * NQT + qt
                                    )
                                )
                    if pending is not None:
                        pending()

                # ===== Phase 3: output projection (wo stationary) =====
                with (
                    tc.tile_pool(name="cx3", bufs=3) as cx3,
                    tc.tile_pool(name="ost", bufs=3) as ost,
                    tc.tile_pool(name="pso", bufs=8, space="PSUM") as pso,
                ):
                    for tg in range(NTG):
                        cxt = cx3.tile([P, HL, 512], BF16, tag="cx")
                        nc.sync.dma_start(cxt, ctx_scr_r[:, :, ts(tg, 512)])
                        for es in range(D // P):
                            ps_o = pso.tile([P, 512], F32, tag="po")
                            for dl in range(HL):
                                nc.tensor.matmul(
                                    ps_o,
                                    wo_sb[:, dl, ts(es, P)],
                                    cxt[:, dl, :],
                                    start=(dl == 0),
                                    stop=(dl == HL - 1),
                                )
                            st = ost.tile([P, 512], BF16, tag="ost")
                            nc.scalar.copy(st, ps_o)
                            nc.sync.dma_start(
                                outT_r[:, es, ts(tg, 512)], st
                            )

    nc.finalize()
    return nc


def kernel(x, wq, wk, wv, wo, cos, sin, mask):
    B, S, D = x.shape
    H = D // P
    NCORES = 8
    HL = H // NCORES
    DL = HL * P
    T = B * S

    import ml_dtypes

    BF = ml_dtypes.bfloat16

    x = np.asarray(x, dtype=np.float32)
    xT = np.ascontiguousarray(x.reshape(T, D).T).astype(BF)
    cos = np.asarray(cos, dtype=np.float32)
    sin = np.asarray(sin, dtype=np.float32)

    # rope tables on [hd-partition, token-free] layout
    cosE = np.repeat(cos.T, 2, axis=0).astype(BF)          # [128, S]
    sinS = np.empty((P, S), dtype=np.float32)              # signed sin
    sinS[0::2] = -sin.T
    sinS[1::2] = sin.T
    sinS = sinS.astype(BF)
    pswap = np.zeros((P, P), dtype=np.float32)
    for r in range(P):
        pswap[r, r ^ 1] = 1.0
    pswap = pswap.astype(BF)

    maskT = np.ascontiguousarray(np.asarray(mask, dtype=np.float32)[0, 0].T)
    kinds, mixed, mixed_index = _classify_blocks(maskT, S, 512)

    nc = _build(B, S, D, HL, kinds, mixed_index, len(mixed))

    wq = np.asarray(wq, dtype=np.float32)
    wk = np.asarray(wk, dtype=np.float32)
    wv = np.asarray(wv, dtype=np.float32)
    wo = np.asarray(wo, dtype=np.float32)

    in_maps = []
    for c in range(NCORES):
        sl = slice(c * DL, (c + 1) * DL)
        m = {
            "xT": xT,
            "cosE": cosE,
            "sinS": sinS,
            "pswap": pswap,
            "mask_tiles": mixed,
            "wqT": np.ascontiguousarray(wq[sl, :].T).astype(BF),
            "wkT": np.ascontiguousarray(wk[sl, :].T).astype(BF),
            "wvT": np.ascontiguousarray(wv[sl, :].T).astype(BF),
            "woT": np.ascontiguousarray(wo[:, sl].T).astype(BF),
        }
        in_maps.append(m)

    kwargs = {}
    if TRACE:
        kwargs = {"trace": True}
        if TRACE_DIR:
            kwargs["tmpdir"] = TRACE_DIR
    res = run_bass_kernel_spmd(nc, in_maps, core_ids=list(range(NCORES)), **kwargs)
    LAST_RESULT[0] = res

    acc = res.results[0]["outT"].astype(np.float32)
    for c in range(1, NCORES):
        acc += res.results[c]["outT"].astype(np.float32)
    return np.ascontiguousarray(acc.T).reshape(B, S, D)
